# revision 13
# baseline (speedup 1.0000x reference)
"""Trainium2 Bass/Tile kernel for nn_Capsule_6004364280312.

CapsNet dynamic routing:
    u_hat = einsum('bnd,dm->bnm', u_vecs, W[0]) reshaped to [B, NC, N, DC]
    3 rounds of routing (softmax over N / weighted sum / squash / agreement)
    returns v [B, NC, DC]

Strategy (per core, batch-parallel over 8 cores, 4 batches each):
  * never materialize u_hat. With e = exp(b) (softmax normalizer cancels
    under the final normalize):
        cuT[d, p] = sum_j u[bl(p)][j, d] * e[p, j]        (matmul, bf16)
        s[p, d']  = sum_d cu[p, d] * W[d, i(p)*64 + d']   (masked matmul)
        b[p, j]  += sum_d u[bl(p)][j, d] * (W_i v)[d, p]  (agreement)
  * partition layout p = bl*32 + i (bl = local batch 0..3, i = capsule).
  * s is computed DIRECTLY in [p, 64] psum: accumulate 64 matmuls whose
    lhsT is cuT scattered block-diagonally over (dk, i) tiles (cuTm) with
    only capsule-i partitions' columns nonzero, rhs = W[:, i*64:(i+1)*64].
    No DRAM bounce / diagonal extraction needed.
  * all matmuls in bf16 (1 cycle/row on PE); accumulation is fp32 in PSUM.
  * squash scale rv = 1/sqrt(||s||^2 + eps) entirely on DVE (Quake seed +
    2 Newton steps) so ACT only ever runs Exp/Copy -> zero activation
    table reloads after the initial one.
  * b lives in PSUM across rounds (buT_ps accumulates with start=False),
    agreement matmuls are 32-col slices per (bl, jc, dk) against unmasked
    wvT column slices.
  * 7 DMAs total: 6 input loads + 1 output store.
"""

import numpy as np
from contextlib import ExitStack

import concourse.bass as bass
import concourse.mybir as mybir
import concourse.tile as tile
from concourse import bacc, bass_utils
from concourse.masks import make_identity

F32 = mybir.dt.float32
I32 = mybir.dt.int32
BF16 = mybir.dt.bfloat16
AF = mybir.ActivationFunctionType
ALU = mybir.AluOpType

B, N, D = 32, 1024, 256
NC, DC = 32, 64
M = NC * DC  # 2048
N_CORES = 8
BL = B // N_CORES  # local batches per core
P = 128
EPS = 1e-7
ROUTINGS = 3
MAGIC = 0x5F3759DF  # Quake fast inverse sqrt seed


def _ap(base, offset, dims):
    """Raw strided AP over the same tensor as `base` (flat element space)."""
    return bass.AP(tensor=base.tensor, offset=offset, ap=dims)


def _build_kernel():
    nc = bacc.Bacc("TRN2", target_bir_lowering=False, debug=False,
                   num_devices=N_CORES)
    u_d = nc.dram_tensor("u", (BL * N, D), F32, kind="ExternalInput").ap()
    w_d = nc.dram_tensor("w", (D, M), F32, kind="ExternalInput").ap()
    v_d = nc.dram_tensor("v", (P, DC), F32, kind="ExternalOutput").ap()

    with tile.TileContext(nc) as tc:
        with ExitStack() as ctx:
            _body(ctx, tc, v_d, u_d, w_d)
    nc.compile()
    return nc


def _body(ctx, tc, v_d, u_d, w_d):
    nc = tc.nc
    const = ctx.enter_context(tc.tile_pool(name="const", bufs=1))
    work = ctx.enter_context(tc.tile_pool(name="work", bufs=2))
    stage = ctx.enter_context(tc.tile_pool(name="stage", bufs=2))
    ustage = ctx.enter_context(tc.tile_pool(name="ustage", bufs=4))
    pq = ctx.enter_context(tc.tile_pool(name="pq", bufs=1, space="PSUM"))
    pmm = ctx.enter_context(tc.tile_pool(name="pmm", bufs=1, space="PSUM"))
    psm = ctx.enter_context(tc.tile_pool(name="psm", bufs=1, space="PSUM"))
    pacc = ctx.enter_context(tc.tile_pool(name="pacc", bufs=1, space="PSUM"))

    # ---------------- constants / persistent SBUF ----------------
    ident = const.tile([P, P], F32)
    make_identity(nc, ident)
    ident_b = const.tile([P, P], BF16)
    nc.gpsimd.tensor_copy(out=ident_b[:], in_=ident[:])
    magic_sb = const.tile([P, 1], I32)
    nc.gpsimd.memset(magic_sb[:], MAGIC)
    c15 = const.tile([P, 1], F32)
    nc.gpsimd.memset(c15[:], 1.5)

    # all-ones rhs for round 0's uniform (unnormalized) softmax
    ones32 = const.tile([P, 32], BF16)
    nc.gpsimd.memset(ones32[:], 1.0)

    u_sb = const.tile([P, BL * 8 * D], BF16)   # u[bl][jk]: [128(j), 256(d)]
    uT_sb = const.tile([P, BL * 2 * N], BF16)  # uT[bl][dk]: [128(d), 1024(j)]
    wbf = const.tile([P, 2 * M], BF16)         # w[dk]: [128(d), 2048(m)]
    wT_sb = const.tile([P, 16 * D], BF16)      # wT[mk]: [128(m), 256(d)]
    # compact exp(b)^T: eTm[j_local, bl*256 + jk*32 + i] = e[bl*32+i, jk*128+j_local]
    # (same column layout as buT_ps/bT_sb, so exp is a plain slice->slice op)
    eTm = const.tile([P, BL * 8 * 32], BF16)
    # cuT scattered block-diagonally: cuTm[(dk,i)][d_local, p] = cuT[dk][d_local,p]
    # for p with capsule i(p) == i, else 0
    cuTm = const.tile([P, 64 * P], BF16)
    nc.gpsimd.memset(cuTm[:], 0.0)
    vemb = const.tile([P, 16 * P], BF16)       # block-diag s embedding
    nc.gpsimd.memset(vemb[:], 0.0)
    wvT_sb = const.tile([P, D], BF16)          # (W_i v)^T: [128(d), (dk,p)]

    # per-round agreement increments dbT[j, (bl, jc, i)] in PSUM; the
    # routing weights accumulate MULTIPLICATIVELY in eTm:
    #     e_r = e_{r-1} * exp(dbT_r)   (== exp(sum dbT) = exp(b))
    # so no f32 b accumulator is needed. bl regions are interleaved across
    # the two PSUM banks so the per-bl exp reads don't false-conflict with
    # the next bl's matmul group (bank-level WAR).
    buT_ps = []
    for _bl in range(BL):
        bu_t = pacc.tile([P, 256], F32, tag=f"bu{_bl}", name=f"bu{_bl}")
        buT_ps.append(bu_t)

    # ------- loads: W then u; bf16 copies split across DVE/ACT -------
    wst_t = []
    for dk in range(2):
        wst = stage.tile([P, M], F32, tag="wst")
        (nc.sync.dma_start if dk == 0 else nc.scalar.dma_start)(
            out=wst[:], in_=w_d[dk * 128:(dk + 1) * 128, :])
        nc.vector.tensor_copy(out=wbf[:, dk * M:dk * M + 1024],
                              in_=wst[:, 0:1024])
        nc.scalar.copy(out=wbf[:, dk * M + 1024:(dk + 1) * M],
                       in_=wst[:, 1024:2048])
        wst_t.append(wst)
    for bl in range(BL):
        ust = ustage.tile([P, 8 * D], F32, tag="ust")
        # gather the 8 j-tiles of batch bl in one DMA:
        # dst[p, (jk, d)] = u[bl*1024 + jk*128 + p, d]
        srcu = _ap(u_d, bl * N * D, [[D, P], [P * D, 8], [1, D]])
        (nc.sync.dma_start if bl % 2 == 0 else nc.scalar.dma_start)(
            out=ust[:].rearrange("p (jk d) -> p jk d", jk=8), in_=srcu)
        nc.vector.tensor_copy(out=u_sb[:, bl * 2048:bl * 2048 + 1024],
                              in_=ust[:, 0:1024])
        nc.scalar.copy(out=u_sb[:, bl * 2048 + 1024:(bl + 1) * 2048],
                       in_=ust[:, 1024:2048])

    copy_engines = [nc.scalar.copy, nc.vector.tensor_copy]

    def emit_w_transposes():
        # W^T: for fixed dk the 16 mk-blocks are stride-256 in wT
        for dk in range(2):
            for g in range(4):
                pt = pq.tile([P, 4 * P], BF16, tag="quad")
                for q in range(4):
                    mk = g * 4 + q
                    nc.tensor.transpose(
                        out=pt[:, q * P:(q + 1) * P],
                        in_=wbf[:, dk * M + mk * 128:dk * M + (mk + 1) * 128],
                        identity=ident_b[:])
                dst = _ap(wT_sb[:], (g * 4) * D + dk * 128,
                          [[16 * D, P], [D, 4], [1, P]])
                copy_engines[(dk * 4 + g) % 2](
                    out=dst, in_=pt[:].rearrange("p (q c) -> p q c", q=4))

    def emit_u_transposes(bl):
        for dk in range(2):
            for g in range(2):
                pt = pq.tile([P, 4 * P], BF16, tag="quad")
                for q in range(4):
                    jk = g * 4 + q
                    nc.tensor.transpose(
                        out=pt[:, q * P:(q + 1) * P],
                        in_=u_sb[:, (bl * 8 + jk) * D + dk * 128:
                                 (bl * 8 + jk) * D + (dk + 1) * 128],
                        identity=ident_b[:])
                eng = (copy_engines[(dk * 2 + g) % 2] if bl == BL - 1
                       else nc.vector.tensor_copy)
                eng(out=uT_sb[:, (bl * 2 + dk) * N + g * 512:
                              (bl * 2 + dk) * N + (g + 1) * 512],
                    in_=pt[:])

    emit_w_transposes()

    # ---------------- routing rounds ----------------
    for r in range(ROUTINGS):
        last_round = (r == ROUTINGS - 1)

        # cuT[d, p=bl*32+i] = sum_jk u[bl,jk][j, d]^T @ eT slice [j, 32]
        # 32-col output slices per (bl, dk) -> no masking, 4x less PE work.
        # dk0/dk1 accumulation groups are open simultaneously so they live
        # in different PSUM banks (a start=True in a bank wipes open
        # partials there): dk0 at cols bl*32, dk1 at cols 512+bl*32. The
        # bl-groups within a bank are sequential (close before next opens).
        # wv reuses cols 128:384 of bank A afterwards (sequential = fine).
        mm_ps = pmm.tile([P, 2 * 512], F32, tag="mm")
        cuT_ps = mm_ps
        for bl in range(BL):
            for jk in range(8):
                rhs = (ones32[:] if r == 0 else
                       eTm[:, bl * 256 + jk * 32:bl * 256 + (jk + 1) * 32])
                for dk in range(2):
                    nc.tensor.matmul(
                        out=cuT_ps[:, dk * 512 + bl * 32:dk * 512 + (bl + 1) * 32],
                        lhsT=u_sb[:, (bl * 8 + jk) * D + dk * 128:
                                  (bl * 8 + jk) * D + (dk + 1) * 128],
                        rhs=rhs,
                        start=(jk == 0), stop=(jk == 7))
            if r == 0 and bl < BL - 1:
                # overlap the u transposes with the load-gated cu matmuls
                emit_u_transposes(bl)

        # scatter cuT into the block-diagonal masked lhsT layout (one copy)
        csrc = _ap(cuT_ps[:], 0, [[1024, P], [512, 2], [32, BL], [1, 32]])
        cdst = _ap(cuTm[:], 0, [[64 * P, P], [32 * P, 2], [32, BL], [P + 1, 32]])
        nc.vector.tensor_copy(out=cdst, in_=csrc)

        # s[p, d'] = sum_{dk,i} cuTm[(dk,i)][:, p]^T @ W[dk-block, i*64+d']
        s_ps = psm.tile([P, DC], F32, tag="s")
        for dk in range(2):
            for i in range(NC):
                nc.tensor.matmul(
                    out=s_ps[:],
                    lhsT=cuTm[:, (dk * NC + i) * P:(dk * NC + i + 1) * P],
                    rhs=wbf[:, dk * M + i * DC:dk * M + (i + 1) * DC],
                    start=(dk == 0 and i == 0),
                    stop=(dk == 1 and i == NC - 1))
        if r == 0:
            # bl3's u transposes: only needed by the agreement matmuls, so
            # emit them here — PE does them during the squash/vemb hops and
            # the DVE/ACT quad copies don't delay the cuTm scatter above
            emit_u_transposes(BL - 1)

        if not last_round:
            # s2 = [s, s] duplicated along free dim, transposed, scattered
            # into the block-diagonal embedding vemb:
            # vemb_k[t*64+d', p] = s[p, d'] for p with capsule i(p) == 2k+t
            # (emitted BEFORE the DVE newton chain so the scatters don't
            # queue behind it — DVE executes in emission order)
            s2_sb = work.tile([P, 2 * DC], BF16, tag="s2")
            nc.scalar.copy(out=s2_sb[:].rearrange("p (t c) -> p t c", t=2),
                           in_=s_ps[:].unsqueeze(1).to_broadcast([P, 2, DC]))
            ptq = pq.tile([P, 4 * P], BF16, tag="quad")
            pt2 = ptq[:, 0:P]
            nc.tensor.transpose(out=pt2[:], in_=s2_sb[:], identity=ident_b[:])
            for t in range(2):
                # ptq is [P, 4*P] so one partition step = 4*P elements
                srcv = _ap(ptq[:], t * 64 * (4 * P) + t,
                           [[4 * P, 64], [2, 16], [32, 4]])
                dstv = _ap(vemb[:], t * 64 * (16 * P) + t,
                           [[16 * P, 64], [P + 2, 16], [32, 4]])
                nc.vector.tensor_copy(out=dstv, in_=srcv)

        # squash scale rv = 1/sqrt(sum(s^2) + eps), entirely on DVE
        # (Quake seed + 2 Newton steps; keeps ACT's table on Exp)
        sq_sb = work.tile([P, DC], F32, tag="sq")
        ssq = work.tile([P, 1], F32, tag="ssq")
        # Square is in every ACT table set -> no table reload; accum_out
        # gives sum(s^2) in one op (PSUM may only feed ONE non-scalar input,
        # so an elementwise s*s on DVE is not allowed here)
        nc.scalar.activation(out=sq_sb[:], in_=s_ps[:], func=AF.Square,
                             accum_out=ssq[:])
        xe = work.tile([P, 1], F32, tag="xe")
        nc.vector.tensor_scalar(out=xe[:], in0=ssq[:], scalar1=EPS,
                                scalar2=None, op0=ALU.add)
        ti = work.tile([P, 1], I32, tag="ti")
        nc.vector.tensor_scalar(out=ti[:], in0=xe[:].bitcast(I32), scalar1=1,
                                scalar2=None, op0=ALU.logical_shift_right)
        y = work.tile([P, 1], F32, tag="y")
        nc.vector.scalar_tensor_tensor(out=y[:].bitcast(I32), in0=magic_sb[:],
                                       scalar=0, in1=ti[:], op0=ALU.bypass,
                                       op1=ALU.subtract)
        for it in range(2 if last_round else 1):
            a = work.tile([P, 1], F32, tag=f"nta{it}")
            nc.vector.scalar_tensor_tensor(out=a[:], in0=y[:], scalar=0.5,
                                           in1=xe[:], op0=ALU.mult,
                                           op1=ALU.mult)
            bq = work.tile([P, 1], F32, tag=f"ntb{it}")
            nc.vector.scalar_tensor_tensor(out=bq[:], in0=y[:], scalar=0.0,
                                           in1=a[:], op0=ALU.bypass,
                                           op1=ALU.mult)
            cq = work.tile([P, 1], F32, tag=f"ntc{it}")
            nc.vector.scalar_tensor_tensor(out=cq[:], in0=bq[:], scalar=-1.0,
                                           in1=c15[:], op0=ALU.mult,
                                           op1=ALU.add)
            y2 = work.tile([P, 1], F32, tag=f"nty{it}")
            nc.vector.tensor_tensor(out=y2[:], in0=y[:], in1=cq[:],
                                    op=ALU.mult)
            y = y2

        if last_round:
            v_sb = work.tile([P, DC], F32, tag="v")
            nc.vector.tensor_scalar(out=v_sb[:], in0=s_ps[:],
                                    scalar1=y[:, 0:1], scalar2=None,
                                    op0=ALU.mult)
            nc.sync.dma_start(out=v_d[:], in_=v_sb[:])
            continue

        # w_v[p, d] = sum_{d'} s[p, d'] * W[d, i(p)*64+d']
        wv_ps = mm_ps[:, P:P + D]
        for k in range(16):
            nc.tensor.matmul(out=wv_ps[:],
                             lhsT=vemb[:, k * P:(k + 1) * P],
                             rhs=wT_sb[:, k * D:(k + 1) * D],
                             start=(k == 0), stop=(k == 15))
        # scale by rv while copying out of psum
        wv_sb = work.tile([P, D], BF16, tag="wvs")
        nc.vector.tensor_scalar(out=wv_sb[:], in0=wv_ps[:],
                                scalar1=y[:, 0:1], scalar2=None, op0=ALU.mult)
        # transpose wv -> wvT[d_local, dk*128 + p]
        ptwq = pq.tile([P, 4 * P], BF16, tag="quad")
        ptw = ptwq[:, 0:2 * P]
        for dk in range(2):
            nc.tensor.transpose(out=ptw[:, dk * P:(dk + 1) * P],
                                in_=wv_sb[:, dk * 128:(dk + 1) * 128],
                                identity=ident_b[:])
        nc.vector.tensor_copy(out=wvT_sb[:], in_=ptw[:])

        # bT[j, (bl, jc-block, i)] += sum_d uT[bl,dk][d, j] * wvT[d, p(bl,i)]
        # fresh psum groups per round; accumulate across rounds in bT_sb,
        # pipelined per bl: matmuls -> DVE add/copy -> ACT exp
        for bl in range(BL):
            bu = buT_ps[bl]
            for jc in range(8):
                for dk in range(2):
                    nc.tensor.matmul(
                        out=bu[:, jc * 32:(jc + 1) * 32],
                        lhsT=uT_sb[:, (bl * 2 + dk) * N + jc * 128:
                                   (bl * 2 + dk) * N + (jc + 1) * 128],
                        rhs=wvT_sb[:, dk * 128 + bl * 32:dk * 128 + (bl + 1) * 32],
                        start=(dk == 0), stop=(dk == 1))
            bsl = slice(bl * 256, (bl + 1) * 256)
            if r == 0:
                nc.scalar.activation(out=eTm[:, bsl], in_=bu[:],
                                     func=AF.Exp)
            else:
                expd = work.tile([P, 256], BF16, tag="expd")
                nc.scalar.activation(out=expd[:], in_=bu[:],
                                     func=AF.Exp)
                nc.vector.tensor_tensor(out=eTm[:, bsl], in0=eTm[:, bsl],
                                        in1=expd[:], op=ALU.mult)


_NC_CACHE = None


def _get_nc():
    global _NC_CACHE
    if _NC_CACHE is None:
        _NC_CACHE = _build_kernel()
    return _NC_CACHE


def kernel(u_vecs: np.ndarray, W: np.ndarray) -> np.ndarray:
    u_vecs = np.ascontiguousarray(np.asarray(u_vecs, dtype=np.float32))
    W0 = np.ascontiguousarray(np.asarray(W, dtype=np.float32).reshape(D, M))
    nc = _get_nc()
    in_maps = [
        {"u": u_vecs[c * BL:(c + 1) * BL].reshape(BL * N, D), "w": W0}
        for c in range(N_CORES)
    ]
    res = bass_utils.run_bass_kernel_spmd(nc, in_maps,
                                          core_ids=list(range(N_CORES)))
    out = np.empty((B, NC, DC), dtype=np.float32)
    for c in range(N_CORES):
        out[c * BL:(c + 1) * BL] = res.results[c]["v"].reshape(BL, NC, DC)
    return out


# revision 14
# speedup vs baseline: 1.2347x; 1.2347x over previous
"""Trainium2 Bass/Tile kernel for nn_Capsule_6004364280312.

CapsNet dynamic routing:
    u_hat = einsum('bnd,dm->bnm', u_vecs, W[0]) reshaped to [B, NC, N, DC]
    3 rounds of routing (softmax over N / weighted sum / squash / agreement)
    returns v [B, NC, DC]

Strategy (per core, batch-parallel over 8 cores, 4 batches each):
  * never materialize u_hat. With e = exp(b) (softmax normalizer cancels
    under the final normalize):
        cuT[d, p] = sum_j u[bl(p)][j, d] * e[p, j]        (matmul, bf16)
        s[p, d']  = sum_d cu[p, d] * W[d, i(p)*64 + d']   (masked matmul)
        b[p, j]  += sum_d u[bl(p)][j, d] * (W_i v)[d, p]  (agreement)
  * partition layout p = bl*32 + i (bl = local batch 0..3, i = capsule).
  * s is computed DIRECTLY in [p, 64] psum: accumulate 64 matmuls whose
    lhsT is cuT scattered block-diagonally over (dk, i) tiles (cuTm) with
    only capsule-i partitions' columns nonzero, rhs = W[:, i*64:(i+1)*64].
    No DRAM bounce / diagonal extraction needed.
  * all matmuls in bf16 (1 cycle/row on PE); accumulation is fp32 in PSUM.
  * squash scale rv = 1/sqrt(||s||^2 + eps) entirely on DVE (Quake seed +
    2 Newton steps) so ACT only ever runs Exp/Copy -> zero activation
    table reloads after the initial one.
  * b lives in PSUM across rounds (buT_ps accumulates with start=False),
    agreement matmuls are 32-col slices per (bl, jc, dk) against unmasked
    wvT column slices.
  * 7 DMAs total: 6 input loads + 1 output store.
"""

import numpy as np
from contextlib import ExitStack

import concourse.bass as bass
import concourse.mybir as mybir
import concourse.tile as tile
from concourse import bacc, bass_utils
from concourse.masks import make_identity

F32 = mybir.dt.float32
I32 = mybir.dt.int32
BF16 = mybir.dt.bfloat16
AF = mybir.ActivationFunctionType
ALU = mybir.AluOpType

B, N, D = 32, 1024, 256
NC, DC = 32, 64
M = NC * DC  # 2048
N_CORES = 8
BL = B // N_CORES  # local batches per core
P = 128
EPS = 1e-7
ROUTINGS = 3
MAGIC = 0x5F3759DF  # Quake fast inverse sqrt seed


def _ap(base, offset, dims):
    """Raw strided AP over the same tensor as `base` (flat element space)."""
    return bass.AP(tensor=base.tensor, offset=offset, ap=dims)


def _build_kernel():
    nc = bacc.Bacc("TRN2", target_bir_lowering=False, debug=False,
                   num_devices=N_CORES)
    u_d = nc.dram_tensor("u", (BL * N, D), F32, kind="ExternalInput").ap()
    w_d = nc.dram_tensor("w", (D, M), F32, kind="ExternalInput").ap()
    v_d = nc.dram_tensor("v", (P, DC), F32, kind="ExternalOutput").ap()

    with tile.TileContext(nc) as tc:
        with ExitStack() as ctx:
            _body(ctx, tc, v_d, u_d, w_d)
    nc.compile()
    return nc


def _body(ctx, tc, v_d, u_d, w_d):
    nc = tc.nc
    const = ctx.enter_context(tc.tile_pool(name="const", bufs=1))
    work = ctx.enter_context(tc.tile_pool(name="work", bufs=2))
    stage = ctx.enter_context(tc.tile_pool(name="stage", bufs=2))
    ustage = ctx.enter_context(tc.tile_pool(name="ustage", bufs=4))
    pq = ctx.enter_context(tc.tile_pool(name="pq", bufs=2, space="PSUM"))
    pmm = ctx.enter_context(tc.tile_pool(name="pmm", bufs=1, space="PSUM"))
    pacc = ctx.enter_context(tc.tile_pool(name="pacc", bufs=1, space="PSUM"))

    # ---------------- constants / persistent SBUF ----------------
    ident = const.tile([P, P], F32)
    make_identity(nc, ident)
    ident_b = const.tile([P, P], BF16)
    nc.gpsimd.tensor_copy(out=ident_b[:], in_=ident[:])
    magic_sb = const.tile([P, 1], I32)
    nc.gpsimd.memset(magic_sb[:], MAGIC)
    c15 = const.tile([P, 1], F32)
    nc.gpsimd.memset(c15[:], 1.5)

    # all-ones rhs for round 0's uniform (unnormalized) softmax
    ones32 = const.tile([P, 32], BF16)
    nc.gpsimd.memset(ones32[:], 1.0)

    u_sb = const.tile([P, BL * 8 * D], BF16)   # u[bl][jk]: [128(j), 256(d)]
    uT_sb = const.tile([P, BL * 2 * N], BF16)  # uT[bl][dk]: [128(d), 1024(j)]
    wbf = const.tile([P, 2 * M], BF16)         # w[dk]: [128(d), 2048(m)]
    wT_sb = const.tile([P, 16 * D], BF16)      # wT[mk]: [128(m), 256(d)]
    # compact exp(b)^T: eTm[j_local, bl*256 + jk*32 + i] = e[bl*32+i, jk*128+j_local]
    # (same column layout as buT_ps/bT_sb, so exp is a plain slice->slice op)
    eTm = const.tile([P, BL * 8 * 32], BF16)
    # cuT scattered block-diagonally: cuTm[(dk,i)][d_local, p] = cuT[dk][d_local,p]
    # for p with capsule i(p) == i, else 0
    cuTm = const.tile([P, 64 * P], BF16)
    nc.gpsimd.memset(cuTm[:], 0.0)
    vemb = const.tile([P, 16 * P], BF16)       # block-diag s embedding
    nc.gpsimd.memset(vemb[:], 0.0)
    wvT_sb = const.tile([P, D], BF16)          # (W_i v)^T: [128(d), (dk,p)]

    # per-round agreement increments dbT[j, (bl, jc, i)] in PSUM; the
    # routing weights accumulate MULTIPLICATIVELY in eTm:
    #     e_r = e_{r-1} * exp(dbT_r)   (== exp(sum dbT) = exp(b))
    # so no f32 b accumulator is needed. bl regions are interleaved across
    # the two PSUM banks so the per-bl exp reads don't false-conflict with
    # the next bl's matmul group (bank-level WAR).
    buT_ps = []
    for _bl in range(BL):
        bu_t = pacc.tile([P, 256], F32, tag=f"bu{_bl}", name=f"bu{_bl}")
        buT_ps.append(bu_t)

    # ------- loads: W then u; bf16 copies split across DVE/ACT -------
    wst_t = []
    for dk in range(2):
        wst = stage.tile([P, M], F32, tag="wst")
        (nc.sync.dma_start if dk == 0 else nc.scalar.dma_start)(
            out=wst[:], in_=w_d[dk * 128:(dk + 1) * 128, :])
        nc.vector.tensor_copy(out=wbf[:, dk * M:dk * M + 1024],
                              in_=wst[:, 0:1024])
        nc.scalar.copy(out=wbf[:, dk * M + 1024:(dk + 1) * M],
                       in_=wst[:, 1024:2048])
        wst_t.append(wst)
    for bl in range(BL):
        ust = ustage.tile([P, 8 * D], F32, tag="ust")
        # gather the 8 j-tiles of batch bl in one DMA:
        # dst[p, (jk, d)] = u[bl*1024 + jk*128 + p, d]
        srcu = _ap(u_d, bl * N * D, [[D, P], [P * D, 8], [1, D]])
        (nc.sync.dma_start if bl % 2 == 0 else nc.scalar.dma_start)(
            out=ust[:].rearrange("p (jk d) -> p jk d", jk=8), in_=srcu)
        nc.vector.tensor_copy(out=u_sb[:, bl * 2048:bl * 2048 + 1024],
                              in_=ust[:, 0:1024])
        nc.scalar.copy(out=u_sb[:, bl * 2048 + 1024:(bl + 1) * 2048],
                       in_=ust[:, 1024:2048])

    copy_engines = [nc.scalar.copy, nc.vector.tensor_copy]

    def emit_w_transposes():
        # W^T: for fixed dk the 16 mk-blocks are stride-256 in wT
        for dk in range(2):
            for g in range(4):
                pt = pq.tile([P, 4 * P], BF16, tag="quad")
                for q in range(4):
                    mk = g * 4 + q
                    nc.tensor.transpose(
                        out=pt[:, q * P:(q + 1) * P],
                        in_=wbf[:, dk * M + mk * 128:dk * M + (mk + 1) * 128],
                        identity=ident_b[:])
                dst = _ap(wT_sb[:], (g * 4) * D + dk * 128,
                          [[16 * D, P], [D, 4], [1, P]])
                copy_engines[(dk * 4 + g) % 2](
                    out=dst, in_=pt[:].rearrange("p (q c) -> p q c", q=4))

    def emit_u_transposes(bl):
        for dk in range(2):
            for g in range(2):
                pt = pq.tile([P, 4 * P], BF16, tag="quad")
                for q in range(4):
                    jk = g * 4 + q
                    nc.tensor.transpose(
                        out=pt[:, q * P:(q + 1) * P],
                        in_=u_sb[:, (bl * 8 + jk) * D + dk * 128:
                                 (bl * 8 + jk) * D + (dk + 1) * 128],
                        identity=ident_b[:])
                eng = (copy_engines[(dk * 2 + g) % 2] if bl == BL - 1
                       else nc.vector.tensor_copy)
                eng(out=uT_sb[:, (bl * 2 + dk) * N + g * 512:
                              (bl * 2 + dk) * N + (g + 1) * 512],
                    in_=pt[:])

    emit_w_transposes()

    # ---------------- routing rounds ----------------
    for r in range(ROUTINGS):
        last_round = (r == ROUTINGS - 1)

        # cuT[d, p=bl*32+i] = sum_jk u[bl,jk][j, d]^T @ eT slice [j, 32]
        # 32-col output slices per (bl, dk) -> no masking, 4x less PE work.
        # dk0/dk1 accumulation groups are open simultaneously so they live
        # in different PSUM banks (a start=True in a bank wipes open
        # partials there): dk0 at cols bl*32, dk1 at cols 512+bl*32. The
        # bl-groups within a bank are sequential (close before next opens).
        # wv reuses cols 128:384 of bank A afterwards (sequential = fine).
        mm_ps = pmm.tile([P, 2 * 512], F32, tag="mm")
        cuT_ps = mm_ps
        for bl in range(BL):
            for jk in range(8):
                rhs = (ones32[:] if r == 0 else
                       eTm[:, bl * 256 + jk * 32:bl * 256 + (jk + 1) * 32])
                for dk in range(2):
                    nc.tensor.matmul(
                        out=cuT_ps[:, dk * 512 + bl * 32:dk * 512 + (bl + 1) * 32],
                        lhsT=u_sb[:, (bl * 8 + jk) * D + dk * 128:
                                  (bl * 8 + jk) * D + (dk + 1) * 128],
                        rhs=rhs,
                        start=(jk == 0), stop=(jk == 7))
            if r == 0 and bl < BL - 1:
                # overlap the u transposes with the load-gated cu matmuls
                emit_u_transposes(bl)

        # scatter cuT into the block-diagonal masked lhsT layout (one copy)
        csrc = _ap(cuT_ps[:], 0, [[1024, P], [512, 2], [32, BL], [1, 32]])
        cdst = _ap(cuTm[:], 0, [[64 * P, P], [32 * P, 2], [32, BL], [P + 1, 32]])
        nc.vector.tensor_copy(out=cdst, in_=csrc)

        # s[p, d'] = sum_{dk,i} cuTm[(dk,i)][:, p]^T @ W[dk-block, i*64+d']
        # (lives in mm_ps cols 640:704 — bank B, sequentially after cu-dk1)
        s_ps = mm_ps[:, 640:640 + DC]
        for dk in range(2):
            for i in range(NC):
                nc.tensor.matmul(
                    out=s_ps[:],
                    lhsT=cuTm[:, (dk * NC + i) * P:(dk * NC + i + 1) * P],
                    rhs=wbf[:, dk * M + i * DC:dk * M + (i + 1) * DC],
                    start=(dk == 0 and i == 0),
                    stop=(dk == 1 and i == NC - 1))
        if r == 0:
            # bl3's u transposes: only needed by the agreement matmuls, so
            # emit them here — PE does them during the squash/vemb hops and
            # the DVE/ACT quad copies don't delay the cuTm scatter above
            emit_u_transposes(BL - 1)

        if not last_round:
            # s2 = [s, s] duplicated along free dim, transposed, scattered
            # into the block-diagonal embedding vemb:
            # vemb_k[t*64+d', p] = s[p, d'] for p with capsule i(p) == 2k+t
            # (emitted BEFORE the DVE newton chain so the scatters don't
            # queue behind it — DVE executes in emission order)
            s2_sb = work.tile([P, 2 * DC], BF16, tag="s2")
            nc.scalar.copy(out=s2_sb[:].rearrange("p (t c) -> p t c", t=2),
                           in_=s_ps[:].unsqueeze(1).to_broadcast([P, 2, DC]))
            ptq = pq.tile([P, 4 * P], BF16, tag="quad")
            pt2 = ptq[:, 0:P]
            nc.tensor.transpose(out=pt2[:], in_=s2_sb[:], identity=ident_b[:])
            for t in range(2):
                # ptq is [P, 4*P] so one partition step = 4*P elements
                srcv = _ap(ptq[:], t * 64 * (4 * P) + t,
                           [[4 * P, 64], [2, 16], [32, 4]])
                dstv = _ap(vemb[:], t * 64 * (16 * P) + t,
                           [[16 * P, 64], [P + 2, 16], [32, 4]])
                nc.vector.tensor_copy(out=dstv, in_=srcv)

        # squash scale rv = 1/sqrt(sum(s^2) + eps), entirely on DVE
        # (Quake seed + 2 Newton steps; keeps ACT's table on Exp)
        sq_sb = work.tile([P, DC], F32, tag="sq")
        ssq = work.tile([P, 1], F32, tag="ssq")
        # Square is in every ACT table set -> no table reload; accum_out
        # gives sum(s^2) in one op (PSUM may only feed ONE non-scalar input,
        # so an elementwise s*s on DVE is not allowed here)
        nc.scalar.activation(out=sq_sb[:], in_=s_ps[:], func=AF.Square,
                             accum_out=ssq[:])
        xe = work.tile([P, 1], F32, tag="xe")
        nc.vector.tensor_scalar(out=xe[:], in0=ssq[:], scalar1=EPS,
                                scalar2=None, op0=ALU.add)
        ti = work.tile([P, 1], I32, tag="ti")
        nc.vector.tensor_scalar(out=ti[:], in0=xe[:].bitcast(I32), scalar1=1,
                                scalar2=None, op0=ALU.logical_shift_right)
        y = work.tile([P, 1], F32, tag="y")
        nc.vector.scalar_tensor_tensor(out=y[:].bitcast(I32), in0=magic_sb[:],
                                       scalar=0, in1=ti[:], op0=ALU.bypass,
                                       op1=ALU.subtract)
        for it in range(2 if last_round else 1):
            a = work.tile([P, 1], F32, tag=f"nta{it}")
            nc.vector.scalar_tensor_tensor(out=a[:], in0=y[:], scalar=0.5,
                                           in1=xe[:], op0=ALU.mult,
                                           op1=ALU.mult)
            bq = work.tile([P, 1], F32, tag=f"ntb{it}")
            nc.vector.scalar_tensor_tensor(out=bq[:], in0=y[:], scalar=0.0,
                                           in1=a[:], op0=ALU.bypass,
                                           op1=ALU.mult)
            cq = work.tile([P, 1], F32, tag=f"ntc{it}")
            nc.vector.scalar_tensor_tensor(out=cq[:], in0=bq[:], scalar=-1.0,
                                           in1=c15[:], op0=ALU.mult,
                                           op1=ALU.add)
            y2 = work.tile([P, 1], F32, tag=f"nty{it}")
            nc.vector.tensor_tensor(out=y2[:], in0=y[:], in1=cq[:],
                                    op=ALU.mult)
            y = y2

        if last_round:
            v_sb = work.tile([P, DC], F32, tag="v")
            nc.vector.tensor_scalar(out=v_sb[:], in0=s_ps[:],
                                    scalar1=y[:, 0:1], scalar2=None,
                                    op0=ALU.mult)
            nc.sync.dma_start(out=v_d[:], in_=v_sb[:])
            continue

        # w_v[p, d] = sum_{d'} s[p, d'] * W[d, i(p)*64+d']
        wv_ps = mm_ps[:, P:P + D]
        for k in range(16):
            nc.tensor.matmul(out=wv_ps[:],
                             lhsT=vemb[:, k * P:(k + 1) * P],
                             rhs=wT_sb[:, k * D:(k + 1) * D],
                             start=(k == 0), stop=(k == 15))
        # scale by rv while copying out of psum
        wv_sb = work.tile([P, D], BF16, tag="wvs")
        nc.vector.tensor_scalar(out=wv_sb[:], in0=wv_ps[:],
                                scalar1=y[:, 0:1], scalar2=None, op0=ALU.mult)
        # transpose wv -> wvT[d_local, dk*128 + p]
        ptwq = pq.tile([P, 4 * P], BF16, tag="quad")
        ptw = ptwq[:, 0:2 * P]
        for dk in range(2):
            nc.tensor.transpose(out=ptw[:, dk * P:(dk + 1) * P],
                                in_=wv_sb[:, dk * 128:(dk + 1) * 128],
                                identity=ident_b[:])
        nc.vector.tensor_copy(out=wvT_sb[:], in_=ptw[:])

        # bT[j, (bl, jc-block, i)] += sum_d uT[bl,dk][d, j] * wvT[d, p(bl,i)]
        # fresh psum groups per round; accumulate across rounds in bT_sb,
        # pipelined per bl: matmuls -> DVE add/copy -> ACT exp
        for bl in range(BL):
            bu = buT_ps[bl]
            for jc in range(8):
                for dk in range(2):
                    nc.tensor.matmul(
                        out=bu[:, jc * 32:(jc + 1) * 32],
                        lhsT=uT_sb[:, (bl * 2 + dk) * N + jc * 128:
                                   (bl * 2 + dk) * N + (jc + 1) * 128],
                        rhs=wvT_sb[:, dk * 128 + bl * 32:dk * 128 + (bl + 1) * 32],
                        start=(dk == 0), stop=(dk == 1))
            bsl = slice(bl * 256, (bl + 1) * 256)
            if r == 0:
                nc.scalar.activation(out=eTm[:, bsl], in_=bu[:],
                                     func=AF.Exp)
            else:
                expd = work.tile([P, 256], BF16, tag="expd")
                nc.scalar.activation(out=expd[:], in_=bu[:],
                                     func=AF.Exp)
                nc.vector.tensor_tensor(out=eTm[:, bsl], in0=eTm[:, bsl],
                                        in1=expd[:], op=ALU.mult)


_NC_CACHE = None


def _get_nc():
    global _NC_CACHE
    if _NC_CACHE is None:
        _NC_CACHE = _build_kernel()
    return _NC_CACHE


def kernel(u_vecs: np.ndarray, W: np.ndarray) -> np.ndarray:
    u_vecs = np.ascontiguousarray(np.asarray(u_vecs, dtype=np.float32))
    W0 = np.ascontiguousarray(np.asarray(W, dtype=np.float32).reshape(D, M))
    nc = _get_nc()
    in_maps = [
        {"u": u_vecs[c * BL:(c + 1) * BL].reshape(BL * N, D), "w": W0}
        for c in range(N_CORES)
    ]
    res = bass_utils.run_bass_kernel_spmd(nc, in_maps,
                                          core_ids=list(range(N_CORES)))
    out = np.empty((B, NC, DC), dtype=np.float32)
    for c in range(N_CORES):
        out[c * BL:(c + 1) * BL] = res.results[c]["v"].reshape(BL, NC, DC)
    return out
